# revision 13
# baseline (speedup 1.0000x reference)
"""CrossModalAttention Trainium2 kernel (8-core data parallel, fp8 GEMM).

Math: with seq_len=1, softmax over one key == 1, so each MultiheadAttention
collapses to   att = kv @ Wc.T + bc  with  Wc = Wo @ Wv (256x256) and
bc = bv @ Wo.T + bo, followed by  out = LayerNorm(x + att) * g + b.

Device dataflow (per core, 16384 rows per modality), one 128-row tile:
  - PE: ONE fp8 DoubleRow matmul (K=256 packed as [128,2], 2 k-rows/cycle)
        computes att64 = x_src @ (64*Wc.T)  (PSUM fp32, natural layout)
  - DVE (batched per quad, cross-bank PSUM read): z64 = att64 + 64*(x+bc)
        -> fp16 SBUF
  - DVE: per-tile bn_stats(z64) -> even/odd 6-tuple stats
  - batched merge per super (no bn_aggr): mean64 = (m_e+m_o)/2,
        var64 = (M2_e+M2_o)/256 + (m_e-m_o)^2/4,
        sd64 = sqrt(var64 + 4096 eps) (ACT), rstd64 = 1/sd64,
        nm = -0.5 (m_e+m_o) rstd64
  - ACT: y = Identity(z64*rstd64 + nm) -> fp16 -> DMA out.
Known-broken ops avoided: DVE accum_out (ttr / stt+accum) and any GpSimd
PSUM access or TensorScalarPtr hard-crash the device (NRT_EXEC_UNIT_
UNRECOVERABLE); grouped bn_stats is rejected by the BIR verifier.
Inputs ship twice: transposed fp8 (PE stationary) + natural fp16*64
(residual), with the baseline's 4-way row interleave so all DMA lines
are >=2KB contiguous.
"""

import os
import numpy as np

N_CORES = 8
B = 131072
E = 256
EPS = 1e-5
ROWS = B // N_CORES          # rows per core per modality
SUPER = 2048                 # rows per DMA super-tile (per modality)
N_SUPER = ROWS // SUPER
JS = SUPER // 512            # quad-groups per super (each 4 tiles of 128)
J = ROWS // 512              # quad-groups per core per modality
TPS = JS * 4                 # tiles per super per modality (16)

_PROGRAM_CACHE = {}


def _build_program(generic_gb):
    import concourse.bass as bass
    import concourse.tile as tile
    from concourse import bacc, mybir

    f32 = mybir.dt.float32
    f16 = mybir.dt.float16
    f8 = mybir.dt.float8e4
    AF = mybir.ActivationFunctionType
    OP = mybir.AluOpType
    DR = mybir.MatmulPerfMode.DoubleRow

    nc = bacc.Bacc("TRN2")

    # ---- DRAM I/O ----
    # xT8[mod, k, p, r]: fp8 transposed shard; feature k*128+p, device col r
    # with r = 512j + 128s + p' <-> original row u = 4*(128j+p') + s.
    xT8 = nc.dram_tensor("xT8", [2, 2, 128, ROWS], f8, kind="ExternalInput")
    # w8[mod, k, p, n] = (64*Wc[mod].T)[k*128+p, n]; col 256 = row-sum of w8
    w8 = nc.dram_tensor("w8", [2, 2, 128, 256], f8, kind="ExternalInput")
    # x64[mod, j, p, s, d] = 64*(x+bc)[u, d] natural layout (row u on part p)
    x64 = nc.dram_tensor("x64", [2, J, 128, 4, 256], f16, kind="ExternalInput")
    if generic_gb:
        g = nc.dram_tensor("g", [2, 1, E], f32, kind="ExternalInput")
        b = nc.dram_tensor("b", [2, 1, E], f32, kind="ExternalInput")
    y = nc.dram_tensor("y", [2, J, 128, 4, 256], f16, kind="ExternalOutput")

    # DRAM views (partition-major)
    xT8_v = xT8.rearrange("m c p n -> p m c n")
    w8_v = w8.rearrange("m c p n -> p m c n")
    x64_v = x64.rearrange("m j p s d -> p m j (s d)")
    y_v = y.rearrange("m j p s d -> p m j (s d)")

    C1 = 1.0 / (256.0 * 64.0)    # satt64 -> mean contribution (z units)
    C2 = 1.0 / 256.0             # s2 = sum(z^2) -> E[z^2]

    with tile.TileContext(nc) as tc:
        with (
            tc.tile_pool(name="const", bufs=1) as const_pool,
            tc.tile_pool(name="xin8", bufs=2) as xin8_pool,
            tc.tile_pool(name="xin16", bufs=2) as xin16_pool,
            tc.tile_pool(name="zbuf", bufs=2) as zbuf_pool,
            tc.tile_pool(name="yout", bufs=2) as yout_pool,
            tc.tile_pool(name="stats", bufs=2) as stats_pool,
            tc.tile_pool(name="zps", bufs=2, space="PSUM") as zps_pool,
        ):
            # ---- constants ----
            w8_sb = const_pool.tile([128, 2, 2, 256], f8)   # [p, mod, k, n]
            nc.sync.dma_start(out=w8_sb, in_=w8_v)
            eps_sb = const_pool.tile([128, 1], f32)
            nc.vector.memset(eps_sb, EPS * 4096.0)
            if generic_gb:
                gb_sb = const_pool.tile([128, 2, 2, E], f32)
                for mod in range(2):
                    nc.sync.dma_start(
                        out=gb_sb[:, mod, 0], in_=g[mod].to_broadcast((128, E))
                    )
                    nc.sync.dma_start(
                        out=gb_sb[:, mod, 1], in_=b[mod].to_broadcast((128, E))
                    )

            for sp in range(N_SUPER):
                n0 = sp * SUPER
                j0 = sp * JS
                t0 = sp * TPS
                xT8_sb = xin8_pool.tile([128, 2, 2, SUPER], f8, tag="xin8")
                nc.sync.dma_start(out=xT8_sb, in_=xT8_v[:, :, :, n0:n0 + SUPER])
                x64_sb = xin16_pool.tile([128, 2, JS, 4, 256], f16, tag="xin16")
                for mod in range(2):
                    nc.sync.dma_start(
                        out=x64_sb[:, mod],
                        in_=x64_v[:, mod, j0:j0 + JS, :],
                    )
                z64_sb = zbuf_pool.tile([128, 2, JS, 4, 256], f16, tag="zbuf")
                y_sb = yout_pool.tile([128, 2, JS, 4, 256], f16, tag="yout")
                st = stats_pool.tile([128, 2, JS, 4, 6], f32, tag="st")

                # ---- per-tile matmul; per-quad residual; per-tile sumsq
                for mod in range(2):
                    src = 1 - mod
                    for jl in range(JS):
                        zq = zps_pool.tile([128, 4, 512], f32, tag="zq")
                        for s in range(4):
                            r0 = (jl * 4 + s) * 128
                            nc.tensor.matmul(
                                zq[:, s, 0:256],
                                xT8_sb[:, src, :, r0:r0 + 128],
                                w8_sb[:, mod],
                                start=True, stop=True,
                                perf_mode=DR,
                                skip_group_check=True,
                            )
                        # residual for the whole quad in one DVE op
                        nc.vector.tensor_add(
                            z64_sb[:, mod, jl],
                            zq[:, :, 0:256],
                            x64_sb[:, mod, jl],
                        )
                        for s in range(4):
                            nc.vector.bn_stats(
                                out=st[:, mod, jl, s, :],
                                in_=z64_sb[:, mod, jl, s],
                            )

                # ---- batched stats merge for the whole super ----
                # st = [128, m_e, M2_e, 128, m_o, M2_o] per tile (even/odd)
                # mean64 = (m_e+m_o)/2; var64 = (M2_e+M2_o)/256 + (m_e-m_o)^2/4
                d = stats_pool.tile([128, 2, JS, 4], f32, tag="d")
                nc.vector.scalar_tensor_tensor(
                    out=d, in0=st[:, :, :, :, 1], scalar=1.0,
                    in1=st[:, :, :, :, 4], op0=OP.mult, op1=OP.subtract,
                )
                d2q = stats_pool.tile([128, 2, JS, 4], f32, tag="d2q")
                nc.vector.scalar_tensor_tensor(
                    out=d2q, in0=d, scalar=0.25, in1=d,
                    op0=OP.mult, op1=OP.mult,
                )
                S = stats_pool.tile([128, 2, JS, 4], f32, tag="S")
                nc.vector.tensor_add(
                    S, st[:, :, :, :, 2], st[:, :, :, :, 5],
                )
                var64 = stats_pool.tile([128, 2, JS, 4], f32, tag="var64")
                nc.vector.scalar_tensor_tensor(
                    out=var64, in0=S, scalar=1.0 / 256.0, in1=d2q,
                    op0=OP.mult, op1=OP.add,
                )
                sd64 = stats_pool.tile([128, 2, JS, 4], f32, tag="sd64")
                nc.scalar.activation(
                    out=sd64, in_=var64, func=AF.Sqrt,
                    bias=eps_sb, scale=1.0,
                )
                rstd64 = stats_pool.tile([128, 2, JS, 4], f32, tag="rstd64")
                nc.vector.reciprocal(out=rstd64, in_=sd64)
                msum = stats_pool.tile([128, 2, JS, 4], f32, tag="msum")
                nc.vector.tensor_add(
                    msum, st[:, :, :, :, 1], st[:, :, :, :, 4],
                )
                nm = stats_pool.tile([128, 2, JS, 4], f32, tag="nm")
                nc.vector.scalar_tensor_tensor(
                    out=nm, in0=msum, scalar=-0.5, in1=rstd64,
                    op0=OP.mult, op1=OP.mult,
                )

                # ---- normalize: y = z64*rstd64 + nm  (fp16 out) ----
                for mod in range(2):
                    for jl in range(JS):
                        for s in range(4):
                            t = jl * 4 + s
                            nc.scalar.activation(
                                out=y_sb[:, mod, jl, s],
                                in_=z64_sb[:, mod, jl, s],
                                func=AF.Identity,
                                bias=nm[:, mod, jl, s:s + 1],
                                scale=rstd64[:, mod, jl, s:s + 1],
                            )
                            if generic_gb:
                                nc.vector.tensor_mul(
                                    y_sb[:, mod, jl, s],
                                    y_sb[:, mod, jl, s],
                                    gb_sb[:, mod, 0],
                                )
                                nc.vector.tensor_add(
                                    y_sb[:, mod, jl, s],
                                    y_sb[:, mod, jl, s],
                                    gb_sb[:, mod, 1],
                                )

                # ---- store super-tile ----
                for mod in range(2):
                    nc.sync.dma_start(
                        out=y_v[:, mod, j0:j0 + JS, :],
                        in_=y_sb[:, mod].rearrange("p j s d -> p j (s d)"),
                    )

    nc.finalize()
    return nc


def _get_program(generic_gb):
    key = bool(generic_gb)
    if key not in _PROGRAM_CACHE:
        _PROGRAM_CACHE[key] = _build_program(key)
    return _PROGRAM_CACHE[key]


def _prep_host(audio_embed, text_embed,
               a2t_in_w, a2t_in_b, a2t_out_w, a2t_out_b,
               t2a_in_w, t2a_in_b, t2a_out_w, t2a_out_b,
               ln1_g, ln1_b, ln2_g, ln2_b):
    import ml_dtypes
    f = np.float32
    h = np.float16
    f8 = ml_dtypes.float8_e4m3
    # fold the two projections: att = kv @ (Wo @ Wv).T + (bv @ Wo.T + bo)
    wv_a, bv_a = a2t_in_w[2 * E:], a2t_in_b[2 * E:]
    wv_t, bv_t = t2a_in_w[2 * E:], t2a_in_b[2 * E:]
    wc_a = (a2t_out_w.astype(np.float64) @ wv_a.astype(np.float64))
    wc_t = (t2a_out_w.astype(np.float64) @ wv_t.astype(np.float64))
    bc_a = (bv_a.astype(np.float64) @ a2t_out_w.T.astype(np.float64)
            + a2t_out_b.astype(np.float64)).astype(f)
    bc_t = (bv_t.astype(np.float64) @ t2a_out_w.T.astype(np.float64)
            + t2a_out_b.astype(np.float64)).astype(f)

    generic_gb = not (
        np.all(ln1_g == 1.0) and np.all(ln1_b == 0.0)
        and np.all(ln2_g == 1.0) and np.all(ln2_b == 0.0)
    )

    audio = np.ascontiguousarray(audio_embed, dtype=f)
    text = np.ascontiguousarray(text_embed, dtype=f)

    # w8[mod] = 64*Wc[mod].T chunked to [2, 128, 257] fp8 (col 256 = rowsum)
    w8_all = np.empty((2, 2, 128, 256), f8)
    for mod, wc in enumerate((wc_a, wc_t)):
        wct = (wc.T * 64.0)  # [256 in, 256 out] float64
        for k in range(2):
            w8_all[mod, k] = wct[k * 128:(k + 1) * 128].astype(f8)

    from concurrent.futures import ThreadPoolExecutor

    JC = ROWS // 512

    def shard(c):
        # device col r = 512j + 128s + p <-> original row u = 4(128j+p)+s
        xT8 = np.empty((2, 2, 128, ROWS), f8)
        x64 = np.empty((2, JC, 128, 4, 256), h)
        for mod, x in enumerate((audio, text)):
            bc = bc_a if mod == 0 else bc_t
            xs = x[c * ROWS:(c + 1) * ROWS]
            x4 = xs.reshape(JC, 128, 4, E)                  # [j, p, s, d]
            xt = x4.transpose(3, 0, 2, 1).reshape(E, ROWS)  # [d, (j s p)]
            x8 = xt.astype(f8)
            xT8[mod, 0] = x8[:128]
            xT8[mod, 1] = x8[128:]
            x64[mod] = ((x4 + bc) * 64.0).astype(h)
        return xT8, x64

    with ThreadPoolExecutor(max_workers=8) as ex:
        shards = list(ex.map(shard, range(N_CORES)))

    in_maps = []
    for c in range(N_CORES):
        m = {"xT8": shards[c][0], "x64": shards[c][1], "w8": w8_all}
        if generic_gb:
            m["g"] = np.stack([
                np.ascontiguousarray(ln1_g, dtype=f).reshape(1, E),
                np.ascontiguousarray(ln2_g, dtype=f).reshape(1, E),
            ])
            m["b"] = np.stack([
                np.ascontiguousarray(ln1_b, dtype=f).reshape(1, E),
                np.ascontiguousarray(ln2_b, dtype=f).reshape(1, E),
            ])
        in_maps.append(m)
    return in_maps, generic_gb


def _run(in_maps, generic_gb, trace=False):
    import sys
    if "/opt/trn_rl_repo" not in sys.path:
        sys.path.insert(0, "/opt/trn_rl_repo")
    from concourse.bass_utils import run_bass_kernel_spmd

    nc = _get_program(generic_gb)
    res = run_bass_kernel_spmd(
        nc, in_maps, list(range(N_CORES)), trace=trace,
    )
    return res


def kernel(**inputs):
    import sys
    if "/opt/trn_rl_repo" not in sys.path:
        sys.path.insert(0, "/opt/trn_rl_repo")
    in_maps, generic_gb = _prep_host(**inputs)
    res = _run(in_maps, generic_gb,
               trace=bool(os.environ.get("KERNEL_TRACE")))
    audio_out = np.concatenate(
        [r["y"][0].reshape(ROWS, E) for r in res.results], axis=0
    ).astype(np.float32)
    text_out = np.concatenate(
        [r["y"][1].reshape(ROWS, E) for r in res.results], axis=0
    ).astype(np.float32)
    kernel.last_exec_time_ns = res.exec_time_ns
    kernel.last_results = res
    return (audio_out, text_out)
